# revision 22
# baseline (speedup 1.0000x reference)
"""T5-style multi-head attention (dense_transformer) for 8 Trainium2 cores.

Data-parallel over batch: core c processes hidden_states[c]. No collectives.
Returns (out [B,S,D] f32, position_bias [1,H,S,S] f32) like the reference.

Per-core pipeline (S=2048, D=512, H=8, DK=64, head pairs of 2):
  prologue: Q^T/K^T projections into [dk,s] layout (host supplies hs^T),
            V projected into [s,dk] with a ones column appended (denominator
            rides the attn@V matmul as output row 64).
  attention (per pair, per 1024-wide q-block, per 128-k-chunk):
            scores^T = K^T.T Q^T (row-tiled pair, f32) -> PSUM
            e = exp(scores + mask - SHIFT)      (ACT, psum->sbuf bf16)
            p = e * expg_table_slice            (DVE bf16 2x, Toeplitz bias)
            o[65, q] += [V|1].T p               (PE, accumulate over k chunks)
  norm:     recip of o row 64, broadcast via zero-step DRAM DMA, multiply.
  epilogue: out = outT.T @ Wo (f32), DMA out.
  position_bias output: strided DRAM->DRAM DMA from an f32 Toeplitz table.
"""

import math
import os
import sys

import numpy as np

for _p in ("/opt/trn_rl_repo", "/root/.axon_site/_ro/trn_rl_repo"):
    if os.path.isdir(_p) and _p not in sys.path:
        sys.path.append(_p)

import ml_dtypes  # noqa: E402

B, S, D, H, DK = 8, 2048, 512, 8, 64
NUM_BUCKETS, MAX_DISTANCE = 32, 128
NPAIR = H // 2
SHIFT = -25.0  # uniform logit shift; cancels in softmax, guards exp overflow

_CACHE = {}


# ---------------------------------------------------------------- host tables
def _bucket(rp):
    """T5 bidirectional relative-position bucket, replicating reference.py."""
    nb = NUM_BUCKETS // 2
    ne = nb // 2
    rb = np.where(rp > 0, nb, 0).astype(np.int32)
    rpa = np.abs(rp)
    log_ratio = np.log(np.maximum(rpa, 1).astype(np.float32) / ne) / math.log(
        MAX_DISTANCE / ne
    )
    lsp = ne + (log_ratio * (nb - ne)).astype(np.int32)
    lsp = np.minimum(lsp, nb - 1)
    return rb + np.where(rpa < ne, rpa, lsp).astype(np.int32)


def _gvec(bias_emb):
    """gvec[h, t] = bias value for relative position rp = t-(S-1), t in [0,2S-2]."""
    t = np.arange(2 * S - 1, dtype=np.int32)
    rb = _bucket(t - (S - 1))  # [2S-1]
    return np.ascontiguousarray(bias_emb[rb].T.astype(np.float32))  # [H, 2S-1]


def _host_tables(bias_emb):
    gv = _gvec(bias_emb)  # [H, 2S-1]
    C = 2 * S - 1
    # exp tables, skewed+reversed: T[h][p, y] = exp(gv[h, C + p - y]), width 2S
    P = np.arange(128)[:, None]
    Y = np.arange(2 * S)[None, :]
    idx = C + P - Y
    valid = (idx >= 0) & (idx <= 2 * S - 2)
    idxc = np.clip(idx, 0, 2 * S - 2)
    gexp = np.empty((H // 2, 128, 2, 2 * S), dtype=ml_dtypes.bfloat16)
    for h in range(H):
        gexp[h // 2, :, h % 2] = (np.exp(gv[h][idxc]) * valid).astype(
            ml_dtypes.bfloat16)
    # f32 toeplitz table for the position_bias output: ctab[h][p, y] = gv[h, y-p]
    Y2 = np.arange(2 * S - 1)[None, :]
    idx2 = Y2 - P
    valid2 = idx2 >= 0
    idx2c = np.clip(idx2, 0, 2 * S - 2)
    ctab = np.empty((H, 128, 2 * S - 1), dtype=np.float32)
    for h in range(H):
        ctab[h] = gv[h][idx2c] * valid2
    return gexp, ctab


# ---------------------------------------------------------------- bass program
def _make_tctx():
    """TileContext subclass splitting multi-wait tail drains.

    The axon-client walrus (CoreV3GenImpl setupSyncWait) rejects instructions
    carrying more than one sync wait; Tile's kernel-tail drain accumulates one
    wait per outstanding semaphore. Split into a chain of single-wait drains.
    """
    import bass_rust
    import concourse.tile as tile
    from concourse.vector_clock import ScopedClock

    MAXW = int(os.environ.get("BASS_MAXW", "1"))  # walrus CoreV3 wait limit

    class SplitDrainTileContext(tile.TileContext):
        def _lower_ordered_insts(self, ordered):
            # Split any instruction carrying more than MAXW sync waits into
            # preceding same-engine NOPs each carrying up to MAXW waits.
            for bb_name, insts in ordered.items():
                out = []
                for inst in insts:
                    si = getattr(inst, "sync_info", None)
                    if si is not None and len(si.on_wait) > MAXW:
                        waits = list(si.on_wait)
                        extra = waits[MAXW:]
                        for j in range(0, len(extra), MAXW):
                            nop = bass_rust.InstNoOp(
                                name=f"{inst.name}_xw{j}", ins=[], outs=[])
                            nop.engine = inst.engine
                            nop.sync_info = bass_rust.SyncInfo(
                                on_wait=extra[j:j + MAXW], on_update=[])
                            self.nc.register_instruction(nop, overwrite=True)
                            out.append(nop)
                        inst.sync_info = bass_rust.SyncInfo(
                            on_wait=waits[:MAXW], on_update=list(si.on_update))
                    out.append(inst)
                insts[:] = out
            return super()._lower_ordered_insts(ordered)

        def _drain_and_barrier(self, tick_clock, wait_clock):
            nc = self.nc
            drain_inst = nc.sync.drain()
            wait_clock.add_sem_waits(
                drain_inst.ins, ScopedClock({None: tick_clock.global_clock})
            )
            si = drain_inst.ins.sync_info
            if si is not None and len(si.on_wait) > 1:
                waits = list(si.on_wait)
                drain_inst.ins.sync_info = bass_rust.SyncInfo(
                    on_wait=[waits[0]], on_update=list(si.on_update)
                )
                for w in waits[1:]:
                    d2 = nc.sync.drain()
                    d2.ins.sync_info = bass_rust.SyncInfo(on_wait=[w], on_update=[])

            nc.all_engine_barrier()
            assert self.sems is not None
            popped = nc._tile_sem_poison_stack.pop()
            assert popped is self._sem_poison
            nc.clear_and_free_semaphores(list(self.sems.allocated().values()))
            nc.all_engine_barrier()

    return SplitDrainTileContext


def _build(S_=S):
    import concourse.bass as bass
    from concourse import mybir

    SplitDrainTileContext = _make_tctx()

    f32, bf16 = mybir.dt.float32, mybir.dt.bfloat16
    f16 = mybir.dt.float16
    Exp = mybir.ActivationFunctionType.Exp
    KC = S_ // 128          # k chunks
    WQ = min(512, S_)       # q window (matmul free width)
    NQB = S_ // WQ          # q windows
    NST = S_ // 128         # s tiles (epilogue)

    nc = bass.Bass()
    hsT = nc.dram_tensor("hsT", [D, S_], f16, kind="ExternalInput")
    wq = nc.dram_tensor("wq", [D, D], f16, kind="ExternalInput")
    wk = nc.dram_tensor("wk", [D, D], f16, kind="ExternalInput")
    wv = nc.dram_tensor("wv", [D, D], f16, kind="ExternalInput")
    wo = nc.dram_tensor("wo", [D, D], f16, kind="ExternalInput")
    maskb = nc.dram_tensor("maskb", [128, KC], f32, kind="ExternalInput")
    gexp = nc.dram_tensor("gexp", [NPAIR, 128, 2, 2 * S_], bf16, kind="ExternalInput")
    ctab = nc.dram_tensor("ctab", [128, 2 * S_ - 1], f32, kind="ExternalInput")
    out_d = nc.dram_tensor("out", [S_, D], f32, kind="ExternalOutput")
    pb_d = nc.dram_tensor("pb", [S_, S_], f32, kind="ExternalOutput")

    with SplitDrainTileContext(nc) as tc:
        with (
            tc.tile_pool(name="persist", bufs=1) as pp,
            tc.tile_pool(name="gx", bufs=2) as gxp,
            tc.tile_pool(name="ew", bufs=3) as ew,
            tc.tile_pool(name="ep5", bufs=5) as ep5,
            tc.tile_pool(name="nrm", bufs=2) as nrm,
            tc.tile_pool(name="stp", bufs=2, space="PSUM") as stp,
            tc.tile_pool(name="opp", bufs=2, space="PSUM") as opp,
            tc.tile_pool(name="drp", bufs=2, space="DRAM") as drp,
            tc.tile_pool(name="osb", bufs=3) as osb,
        ):
            mb = pp.tile([128, KC], f32, tag="maskb")
            nc.sync.dma_start(mb[:, :], maskb[:, :])
            wo_sb = []
            for dc in range(4):
                t = pp.tile([128, D], f16, tag=f"wo{dc}", name=f"wo{dc}")
                nc.scalar.dma_start(t[:, :], wo[dc * 128:(dc + 1) * 128, :])
                wo_sb.append(t)

            hs_sb, wq_sb, wk_sb, wv_sb = [], [], [], []
            for dc in range(4):
                t = pp.tile([128, S_], f16, tag=f"hst{dc}", name=f"hst{dc}")
                (nc.sync if dc % 2 else nc.scalar).dma_start(
                    t[:, :], hsT[dc * 128:(dc + 1) * 128, :])
                hs_sb.append(t)
            for wi, (name, dram, lst) in enumerate((
                    ("wq", wq, wq_sb), ("wk", wk, wk_sb), ("wv", wv, wv_sb))):
                for dc in range(4):
                    t = pp.tile([128, D], f16, tag=f"{name}{dc}", name=f"{name}{dc}")
                    (nc.sync if (wi + dc) % 2 else nc.scalar).dma_start(
                        t[:, :], dram[dc * 128:(dc + 1) * 128, :])
                    lst.append(t)

            QT, KT, V, OT = {}, {}, {}, {}

            def project(p):
                """QT/KT [2x64 feats, S] f16, V [s, 2x(64+ones)] bf16 for pair p.
                PSUM tiles share the o0/o1 tags so this work interleaves with
                attention of the previous pair."""
                cs = slice(p * 128, (p + 1) * 128)
                QT[p] = pp.tile([128, S_], f16, tag=f"qt{p}", name=f"qt{p}")
                KT[p] = pp.tile([128, S_], f16, tag=f"kt{p}", name=f"kt{p}")
                V[p] = pp.tile([128, KC * 130], bf16, tag=f"v{p}", name=f"v{p}")
                for dst, w_sb in ((QT[p], wq_sb), (KT[p], wk_sb)):
                    for q5 in range(NQB):
                        qs = slice(q5 * WQ, (q5 + 1) * WQ)
                        ps_t = opp.tile([128, WQ], f32, tag=f"o{q5 % 2}",
                                        name=f"pj{p}")
                        for dc in range(4):
                            nc.tensor.matmul(
                                ps_t[:, :], w_sb[dc][:, cs], hs_sb[dc][:, qs],
                                start=(dc == 0), stop=(dc == 3))
                        nc.vector.tensor_copy(dst[:, qs], ps_t[:, :])
                nc.vector.memset(V[p][:, :], 1.0)
                for kc in range(KC):
                    ks = slice(kc * 128, (kc + 1) * 128)
                    ps_v = opp.tile([128, 128], f32, tag=f"o{kc % 2}",
                                    name=f"pv{p}")
                    for dc in range(4):
                        nc.tensor.matmul(
                            ps_v[:, :], hs_sb[dc][:, ks], wv_sb[dc][:, cs],
                            start=(dc == 0), stop=(dc == 3))
                    nc.vector.tensor_copy(
                        V[p][:, kc * 130:kc * 130 + 130]
                        .rearrange("p (h w) -> p h w", w=65)[:, :, 0:64],
                        ps_v[:, :].rearrange("p (h w) -> p h w", h=2))

            project(0)
            for p in range(NPAIR):
                gx = gxp.tile([128, 2, 2 * S_], bf16, tag="gx", name="gx")
                nc.scalar.dma_start(gx[:, :, :], gexp[p, :, :, :])
                OT[p] = pp.tile([128, S_], f16, tag=f"ot{p}", name=f"ot{p}")
                ost = [pp.tile([64, S_], f32, tag=f"os{hh}", name=f"os{hh}",
                               bufs=2) for hh in range(2)]
                for qb0 in range(0, NQB, 2):
                    qbs = [q for q in (qb0, qb0 + 1) if q < NQB]
                    den = ew.tile([97, WQ], f32, tag="den")
                    nc.vector.memset(den[:, :], 1.0)
                    for qb in qbs:
                        qoff = qb * WQ
                        o = [opp.tile([65, WQ], f32, tag=f"o{hh}",
                                      name=f"o{hh}") for hh in range(2)]
                        pts = {}
                        for kc in range(KC + 1):
                            # produce pt(kc); attn@V lags one chunk so the
                            # PE dequeues next scores before waiting on DVE
                            if kc < KC:
                                ks = slice(kc * 128, (kc + 1) * 128)
                                st = stp.tile([128, 2 * WQ], f32, tag="st")
                                for hh in range(2):
                                    hs_ = slice(hh * 64, hh * 64 + 64)
                                    nc.tensor.matmul(
                                        st[:, hh * WQ:(hh + 1) * WQ],
                                        KT[p][hs_, ks],
                                        QT[p][hs_, qoff:qoff + WQ],
                                        start=True, stop=True)
                                e = ep5.tile([128, 2 * WQ], bf16, tag="e")
                                nc.scalar.activation(e[:, :], st[:, :], Exp,
                                                     bias=mb[:, kc:kc + 1])
                                y0 = S_ - 128 * kc + qoff
                                pt = ep5.tile([128, 2, WQ], bf16, tag="pt")
                                nc.vector.tensor_mul(
                                    pt[:, :, :],
                                    e[:, :].rearrange("p (h w) -> p h w", h=2),
                                    gx[:, :, y0:y0 + WQ])
                                pts[kc] = pt
                            if kc >= 1:
                                kp = kc - 1
                                ptp = pts.pop(kp)
                                for hh in range(2):
                                    nc.tensor.matmul(
                                        o[hh][:, :],
                                        V[p][:, kp * 130 + hh * 65:
                                             kp * 130 + hh * 65 + 65],
                                        ptp[:, hh, :],
                                        start=(kp == 0), stop=(kp == KC - 1))
                        # evacuate: denominator row + unnormalized features
                        for hh in range(2):
                            slot = 32 * (hh * 2 + (qb - qb0))
                            nc.vector.tensor_copy(den[slot:slot + 1, :],
                                                   o[hh][64:65, :])
                            nc.vector.tensor_copy(ost[hh][:, qoff:qoff + WQ],
                                                  o[hh][0:64, :])
                    # batch normalization for these q windows
                    rec = ew.tile([97, WQ], f32, tag="rec")
                    nc.vector.reciprocal(rec[:, :], den[:, :])
                    rdd = drp.tile([4, WQ], f32, tag="rdd")
                    for hh in range(2):
                        for qb in qbs:
                            s_ = hh * 2 + (qb - qb0)
                            nc.sync.dma_start(rdd[s_:s_ + 1, :],
                                              rec[32 * s_:32 * s_ + 1, :])
                    for hh in range(2):
                        for qb in qbs:
                            s_ = hh * 2 + (qb - qb0)
                            qoff = qb * WQ
                            bc = nrm.tile([64, WQ], f32, tag="bc")
                            nc.sync.dma_start(
                                bc[:, :], rdd[s_:s_ + 1, :].partition_broadcast(64))
                            nc.vector.tensor_mul(
                                OT[p][hh * 64:hh * 64 + 64, qoff:qoff + WQ],
                                ost[hh][:, qoff:qoff + WQ], bc[:, :])
                if p + 1 < NPAIR:
                    project(p + 1)

            # epilogue: output projection
            for st_i in range(NST):
                ss = slice(st_i * 128, (st_i + 1) * 128)
                ps_o = opp.tile([128, D], f32, tag=f"o{st_i % 2}", name="eo")
                for p in range(NPAIR):
                    nc.tensor.matmul(ps_o[:, :], OT[p][:, ss], wo_sb[p][:, :],
                                     start=(p == 0), stop=(p == NPAIR - 1))
                ob = osb.tile([128, D], f32, tag="ob")
                nc.scalar.copy(ob[:, :], ps_o[:, :])
                nc.sync.dma_start(out_d[ss, :], ob[:, :])

            # position_bias output: strided DRAM->DRAM expansion (emitted
            # last so it never queues ahead of latency-critical transfers)
            for qi in range(NST):
                a = (S_ - 1) - 128 * qi
                (nc.sync if qi % 2 else nc.scalar).dma_start(
                    pb_d[qi * 128:(qi + 1) * 128, :], ctab[:, a:a + S_])

    nc.finalize()
    return nc


def _get_nc(S_=S):
    if ("nc", S_) not in _CACHE:
        _CACHE[("nc", S_)] = _build(S_)
    return _CACHE[("nc", S_)]


def _ensure_profile_hook():
    """Provide antenv.axon_hooks if the image lacks it, so trace=True can
    capture NTFF profiles (and a stray BASS_TRACE env cannot crash us)."""
    import types

    try:
        from antenv import axon_hooks  # noqa: F401
        return
    except ImportError:
        pass
    hook = None
    try:
        from trn_agent_boot.trn_boot import _ntff_profile_via_ctypes

        hook = _ntff_profile_via_ctypes("/opt/axon/libaxon_pjrt.so")
    except Exception:
        hook = None
    m = types.ModuleType("antenv.axon_hooks")
    m.get_axon_ntff_profile_hook = lambda: hook
    m.set_axon_ntff_profile_hook = lambda h: None
    sys.modules["antenv.axon_hooks"] = m


# ---------------------------------------------------------------- entry point
def kernel(hidden_states, mask, Wq, Wk, Wv, Wo, bias_emb):
    from concourse.bass_utils import run_bass_kernel_spmd

    hidden_states = np.asarray(hidden_states, dtype=np.float32)
    mask = np.asarray(mask, dtype=np.float32)
    Wq = np.ascontiguousarray(np.asarray(Wq, dtype=np.float16))
    Wk = np.ascontiguousarray(np.asarray(Wk, dtype=np.float16))
    Wv = np.ascontiguousarray(np.asarray(Wv, dtype=np.float16))
    Wo = np.ascontiguousarray(np.asarray(Wo, dtype=np.float16))
    bias_emb = np.asarray(bias_emb, dtype=np.float32)

    KC = S // 128
    gexp, ctab = _host_tables(bias_emb)
    nc = _get_nc()

    in_maps = []
    for c in range(B):
        in_maps.append({
            "hsT": np.ascontiguousarray(hidden_states[c].T.astype(np.float16)),
            "wq": Wq, "wk": Wk, "wv": Wv, "wo": Wo,
            "maskb": np.ascontiguousarray(
                mask[c, 0, 0, :].reshape(KC, 128).T + np.float32(SHIFT)),
            "gexp": gexp,
            "ctab": np.ascontiguousarray(ctab[c % H]),
        })

    trace = bool(int(os.environ.get("KERNEL_TRACE", "0")))
    _ensure_profile_hook()
    if trace:
        from concourse import bass_utils as _bu

        _bu.upload_artifacts = lambda tmpdir: tmpdir  # no artifact bucket here
    res = run_bass_kernel_spmd(nc, in_maps, core_ids=list(range(B)), trace=trace,
                               tmpdir=os.environ.get("KERNEL_TRACE_DIR"))
    global LAST_EXEC_NS, LAST_RESULT
    LAST_EXEC_NS = res.exec_time_ns
    LAST_RESULT = res

    out = np.stack([np.asarray(res.results[c]["out"]) for c in range(B)])
    pb = np.stack([np.asarray(res.results[c]["pb"]) for c in range(H)])[None]
    return out, pb


LAST_EXEC_NS = None


# revision 24
# speedup vs baseline: 1.0791x; 1.0791x over previous
"""T5-style multi-head attention (dense_transformer) for 8 Trainium2 cores.

Data-parallel over batch: core c processes hidden_states[c]. No collectives.
Returns (out [B,S,D] f32, position_bias [1,H,S,S] f32) like the reference.

Per-core pipeline (S=2048, D=512, H=8, DK=64, head pairs of 2):
  prologue: Q^T/K^T projections into [dk,s] layout (host supplies hs^T),
            V projected into [s,dk] with a ones column appended (denominator
            rides the attn@V matmul as output row 64).
  attention (per pair, per 1024-wide q-block, per 128-k-chunk):
            scores^T = K^T.T Q^T (row-tiled pair, f32) -> PSUM
            e = exp(scores + mask - SHIFT)      (ACT, psum->sbuf bf16)
            p = e * expg_table_slice            (DVE bf16 2x, Toeplitz bias)
            o[65, q] += [V|1].T p               (PE, accumulate over k chunks)
  norm:     recip of o row 64, broadcast via zero-step DRAM DMA, multiply.
  epilogue: out = outT.T @ Wo (f32), DMA out.
  position_bias output: strided DRAM->DRAM DMA from an f32 Toeplitz table.
"""

import math
import os
import sys

import numpy as np

for _p in ("/opt/trn_rl_repo", "/root/.axon_site/_ro/trn_rl_repo"):
    if os.path.isdir(_p) and _p not in sys.path:
        sys.path.append(_p)

import ml_dtypes  # noqa: E402

B, S, D, H, DK = 8, 2048, 512, 8, 64
NUM_BUCKETS, MAX_DISTANCE = 32, 128
NPAIR = H // 2
SHIFT = -25.0  # uniform logit shift; cancels in softmax, guards exp overflow

_CACHE = {}


# ---------------------------------------------------------------- host tables
def _bucket(rp):
    """T5 bidirectional relative-position bucket, replicating reference.py."""
    nb = NUM_BUCKETS // 2
    ne = nb // 2
    rb = np.where(rp > 0, nb, 0).astype(np.int32)
    rpa = np.abs(rp)
    log_ratio = np.log(np.maximum(rpa, 1).astype(np.float32) / ne) / math.log(
        MAX_DISTANCE / ne
    )
    lsp = ne + (log_ratio * (nb - ne)).astype(np.int32)
    lsp = np.minimum(lsp, nb - 1)
    return rb + np.where(rpa < ne, rpa, lsp).astype(np.int32)


def _gvec(bias_emb):
    """gvec[h, t] = bias value for relative position rp = t-(S-1), t in [0,2S-2]."""
    t = np.arange(2 * S - 1, dtype=np.int32)
    rb = _bucket(t - (S - 1))  # [2S-1]
    return np.ascontiguousarray(bias_emb[rb].T.astype(np.float32))  # [H, 2S-1]


def _host_tables(bias_emb):
    gv = _gvec(bias_emb)  # [H, 2S-1]
    C = 2 * S - 1
    # exp tables, skewed+reversed: T[h][p, y] = exp(gv[h, C + p - y]), width 2S
    P = np.arange(128)[:, None]
    Y = np.arange(2 * S)[None, :]
    idx = C + P - Y
    valid = (idx >= 0) & (idx <= 2 * S - 2)
    idxc = np.clip(idx, 0, 2 * S - 2)
    gexp = np.empty((H // 2, 128, 2, 2 * S), dtype=ml_dtypes.bfloat16)
    for h in range(H):
        gexp[h // 2, :, h % 2] = (np.exp(gv[h][idxc]) * valid).astype(
            ml_dtypes.bfloat16)
    # f32 toeplitz table for the position_bias output: ctab[h][p, y] = gv[h, y-p]
    Y2 = np.arange(2 * S - 1)[None, :]
    idx2 = Y2 - P
    valid2 = idx2 >= 0
    idx2c = np.clip(idx2, 0, 2 * S - 2)
    ctab = np.empty((H, 128, 2 * S - 1), dtype=np.float32)
    for h in range(H):
        ctab[h] = gv[h][idx2c] * valid2
    return gexp, ctab


# ---------------------------------------------------------------- bass program
def _make_tctx():
    """TileContext subclass splitting multi-wait tail drains.

    The axon-client walrus (CoreV3GenImpl setupSyncWait) rejects instructions
    carrying more than one sync wait; Tile's kernel-tail drain accumulates one
    wait per outstanding semaphore. Split into a chain of single-wait drains.
    """
    import bass_rust
    import concourse.tile as tile
    from concourse.vector_clock import ScopedClock

    MAXW = int(os.environ.get("BASS_MAXW", "1"))  # walrus CoreV3 wait limit

    class SplitDrainTileContext(tile.TileContext):
        def _lower_ordered_insts(self, ordered):
            # Split any instruction carrying more than MAXW sync waits into
            # preceding same-engine NOPs each carrying up to MAXW waits.
            for bb_name, insts in ordered.items():
                out = []
                for inst in insts:
                    si = getattr(inst, "sync_info", None)
                    if si is not None and len(si.on_wait) > MAXW:
                        waits = list(si.on_wait)
                        extra = waits[MAXW:]
                        for j in range(0, len(extra), MAXW):
                            nop = bass_rust.InstNoOp(
                                name=f"{inst.name}_xw{j}", ins=[], outs=[])
                            nop.engine = inst.engine
                            nop.sync_info = bass_rust.SyncInfo(
                                on_wait=extra[j:j + MAXW], on_update=[])
                            self.nc.register_instruction(nop, overwrite=True)
                            out.append(nop)
                        inst.sync_info = bass_rust.SyncInfo(
                            on_wait=waits[:MAXW], on_update=list(si.on_update))
                    out.append(inst)
                insts[:] = out
            return super()._lower_ordered_insts(ordered)

        def _drain_and_barrier(self, tick_clock, wait_clock):
            nc = self.nc
            drain_inst = nc.sync.drain()
            wait_clock.add_sem_waits(
                drain_inst.ins, ScopedClock({None: tick_clock.global_clock})
            )
            si = drain_inst.ins.sync_info
            if si is not None and len(si.on_wait) > 1:
                waits = list(si.on_wait)
                drain_inst.ins.sync_info = bass_rust.SyncInfo(
                    on_wait=[waits[0]], on_update=list(si.on_update)
                )
                for w in waits[1:]:
                    d2 = nc.sync.drain()
                    d2.ins.sync_info = bass_rust.SyncInfo(on_wait=[w], on_update=[])

            nc.all_engine_barrier()
            assert self.sems is not None
            popped = nc._tile_sem_poison_stack.pop()
            assert popped is self._sem_poison
            nc.clear_and_free_semaphores(list(self.sems.allocated().values()))
            nc.all_engine_barrier()

    return SplitDrainTileContext


def _build(S_=S):
    import concourse.bass as bass
    from concourse import mybir

    SplitDrainTileContext = _make_tctx()

    f32, bf16 = mybir.dt.float32, mybir.dt.bfloat16
    f16 = mybir.dt.float16
    Exp = mybir.ActivationFunctionType.Exp
    KC = S_ // 128          # k chunks
    WQ = min(512, S_)       # q window (matmul free width)
    NQB = S_ // WQ          # q windows
    NST = S_ // 128         # s tiles (epilogue)

    nc = bass.Bass()
    hsT = nc.dram_tensor("hsT", [D, S_], f16, kind="ExternalInput")
    wq = nc.dram_tensor("wq", [D, D], f16, kind="ExternalInput")
    wk = nc.dram_tensor("wk", [D, D], f16, kind="ExternalInput")
    wv = nc.dram_tensor("wv", [D, D], f16, kind="ExternalInput")
    wo = nc.dram_tensor("wo", [D, D], f16, kind="ExternalInput")
    maskb = nc.dram_tensor("maskb", [128, KC], f32, kind="ExternalInput")
    gexp = nc.dram_tensor("gexp", [NPAIR, 128, 2, 2 * S_], bf16, kind="ExternalInput")
    ctab = nc.dram_tensor("ctab", [128, 2 * S_ - 1], f32, kind="ExternalInput")
    out_d = nc.dram_tensor("out", [S_, D], f32, kind="ExternalOutput")
    pb_d = nc.dram_tensor("pb", [S_, S_], f32, kind="ExternalOutput")

    with SplitDrainTileContext(nc) as tc:
        with (
            tc.tile_pool(name="persist", bufs=1) as pp,
            tc.tile_pool(name="gx", bufs=2) as gxp,
            tc.tile_pool(name="ew", bufs=3) as ew,
            tc.tile_pool(name="ep5", bufs=5) as ep5,
            tc.tile_pool(name="nrm", bufs=2) as nrm,
            tc.tile_pool(name="stp", bufs=2, space="PSUM") as stp,
            tc.tile_pool(name="opp", bufs=2, space="PSUM") as opp,
            tc.tile_pool(name="drp", bufs=2, space="DRAM") as drp,
            tc.tile_pool(name="osb", bufs=3) as osb,
        ):
            mb = pp.tile([128, KC], f32, tag="maskb")
            nc.sync.dma_start(mb[:, :], maskb[:, :])
            wo_sb = []
            for dc in range(4):
                t = pp.tile([128, D], f16, tag=f"wo{dc}", name=f"wo{dc}")
                nc.scalar.dma_start(t[:, :], wo[dc * 128:(dc + 1) * 128, :])
                wo_sb.append(t)

            hs_sb, wq_sb, wk_sb, wv_sb = [], [], [], []
            for dc in range(4):
                t = pp.tile([128, S_], f16, tag=f"hst{dc}", name=f"hst{dc}")
                (nc.sync if dc % 2 else nc.scalar).dma_start(
                    t[:, :], hsT[dc * 128:(dc + 1) * 128, :])
                hs_sb.append(t)
            for wi, (name, dram, lst) in enumerate((
                    ("wq", wq, wq_sb), ("wk", wk, wk_sb), ("wv", wv, wv_sb))):
                for dc in range(4):
                    t = pp.tile([128, D], f16, tag=f"{name}{dc}", name=f"{name}{dc}")
                    (nc.sync if (wi + dc) % 2 else nc.scalar).dma_start(
                        t[:, :], dram[dc * 128:(dc + 1) * 128, :])
                    lst.append(t)

            QT, KT, V, OT = {}, {}, {}, {}

            def project_steps(p):
                """QT/KT [2x64 feats, S] f16, V [s, 2x(64+ones)] bf16 for
                pair p, as a list of small emit-steps so they can interleave
                with the previous pair's attention (PSUM via o0/o1 tags)."""
                cs = slice(p * 128, (p + 1) * 128)
                QT[p] = pp.tile([128, S_], f16, tag=f"qt{p}", name=f"qt{p}")
                KT[p] = pp.tile([128, S_], f16, tag=f"kt{p}", name=f"kt{p}")
                V[p] = pp.tile([128, KC * 130], bf16, tag=f"v{p}", name=f"v{p}")
                steps = []

                def memset_step():
                    nc.vector.memset(V[p][:, :], 1.0)
                steps.append(memset_step)

                def qk_step(dst, w_sb, q5):
                    qs = slice(q5 * WQ, (q5 + 1) * WQ)
                    ps_t = opp.tile([128, WQ], f32, tag=f"o{q5 % 2}",
                                    name=f"pj{p}")
                    for dc in range(4):
                        nc.tensor.matmul(
                            ps_t[:, :], w_sb[dc][:, cs], hs_sb[dc][:, qs],
                            start=(dc == 0), stop=(dc == 3))
                    nc.vector.tensor_copy(dst[:, qs], ps_t[:, :])

                for q5 in range(NQB):
                    steps.append(lambda q5=q5: qk_step(QT[p], wq_sb, q5))
                    steps.append(lambda q5=q5: qk_step(KT[p], wk_sb, q5))

                def v_step(kc):
                    ks = slice(kc * 128, (kc + 1) * 128)
                    ps_v = opp.tile([128, 128], f32, tag=f"o{kc % 2}",
                                    name=f"pv{p}")
                    for dc in range(4):
                        nc.tensor.matmul(
                            ps_v[:, :], hs_sb[dc][:, ks], wv_sb[dc][:, cs],
                            start=(dc == 0), stop=(dc == 3))
                    nc.vector.tensor_copy(
                        V[p][:, kc * 130:kc * 130 + 130]
                        .rearrange("p (h w) -> p h w", w=65)[:, :, 0:64],
                        ps_v[:, :].rearrange("p (h w) -> p h w", h=2))

                for kc in range(KC):
                    steps.append(lambda kc=kc: v_step(kc))
                return steps

            for step in project_steps(0):
                step()
            pending_proj = []
            for p in range(NPAIR):
                gx = gxp.tile([128, 2, 2 * S_], bf16, tag="gx", name="gx")
                nc.sync.dma_start(gx[:, :, :], gexp[p, :, :, :])
                OT[p] = pp.tile([128, S_], f16, tag=f"ot{p}", name=f"ot{p}")
                if p + 1 < NPAIR:
                    pending_proj.extend(project_steps(p + 1))
                ost = [pp.tile([64, S_], f32, tag=f"os{hh}", name=f"os{hh}",
                               bufs=2) for hh in range(2)]
                for qb0 in range(0, NQB, 2):
                    qbs = [q for q in (qb0, qb0 + 1) if q < NQB]
                    den = ew.tile([97, WQ], f32, tag="den")
                    nc.vector.memset(den[:, :], 1.0)
                    for qb in qbs:
                        qoff = qb * WQ
                        o = [opp.tile([65, WQ], f32, tag=f"o{hh}",
                                      name=f"o{hh}") for hh in range(2)]
                        pts = {}
                        for kc in range(KC + 1):
                            # produce pt(kc); attn@V lags one chunk so the
                            # PE dequeues next scores before waiting on DVE
                            if kc < KC:
                                ks = slice(kc * 128, (kc + 1) * 128)
                                st = stp.tile([128, 2 * WQ], f32, tag="st")
                                for hh in range(2):
                                    hs_ = slice(hh * 64, hh * 64 + 64)
                                    nc.tensor.matmul(
                                        st[:, hh * WQ:(hh + 1) * WQ],
                                        KT[p][hs_, ks],
                                        QT[p][hs_, qoff:qoff + WQ],
                                        start=True, stop=True)
                                e = ep5.tile([128, 2 * WQ], bf16, tag="e")
                                nc.scalar.activation(e[:, :], st[:, :], Exp,
                                                     bias=mb[:, kc:kc + 1])
                                y0 = S_ - 128 * kc + qoff
                                pt = ep5.tile([128, 2, WQ], bf16, tag="pt")
                                nc.vector.tensor_mul(
                                    pt[:, :, :],
                                    e[:, :].rearrange("p (h w) -> p h w", h=2),
                                    gx[:, :, y0:y0 + WQ])
                                pts[kc] = pt
                            if kc % 2 == 1 and pending_proj:
                                pending_proj.pop(0)()
                            if kc >= 1:
                                kp = kc - 1
                                ptp = pts.pop(kp)
                                for hh in range(2):
                                    nc.tensor.matmul(
                                        o[hh][:, :],
                                        V[p][:, kp * 130 + hh * 65:
                                             kp * 130 + hh * 65 + 65],
                                        ptp[:, hh, :],
                                        start=(kp == 0), stop=(kp == KC - 1))
                        # evacuate: denominator row + unnormalized features
                        for hh in range(2):
                            slot = 32 * (hh * 2 + (qb - qb0))
                            nc.vector.tensor_copy(den[slot:slot + 1, :],
                                                   o[hh][64:65, :])
                            nc.vector.tensor_copy(ost[hh][:, qoff:qoff + WQ],
                                                  o[hh][0:64, :])
                    # batch normalization for these q windows
                    rec = ew.tile([97, WQ], f32, tag="rec")
                    nc.vector.reciprocal(rec[:, :], den[:, :])
                    rdd = drp.tile([4, WQ], f32, tag="rdd")
                    for hh in range(2):
                        for qb in qbs:
                            s_ = hh * 2 + (qb - qb0)
                            nc.sync.dma_start(rdd[s_:s_ + 1, :],
                                              rec[32 * s_:32 * s_ + 1, :])
                    while pending_proj:
                        pending_proj.pop(0)()
                    for hh in range(2):
                        for qb in qbs:
                            s_ = hh * 2 + (qb - qb0)
                            qoff = qb * WQ
                            bc = nrm.tile([64, WQ], f32, tag="bc")
                            nc.sync.dma_start(
                                bc[:, :], rdd[s_:s_ + 1, :].partition_broadcast(64))
                            nc.vector.tensor_mul(
                                OT[p][hh * 64:hh * 64 + 64, qoff:qoff + WQ],
                                ost[hh][:, qoff:qoff + WQ], bc[:, :])


            # epilogue: output projection
            for st_i in range(NST):
                ss = slice(st_i * 128, (st_i + 1) * 128)
                ps_o = opp.tile([128, D], f32, tag=f"o{st_i % 2}", name="eo")
                for p in range(NPAIR):
                    nc.tensor.matmul(ps_o[:, :], OT[p][:, ss], wo_sb[p][:, :],
                                     start=(p == 0), stop=(p == NPAIR - 1))
                ob = osb.tile([128, D], f32, tag="ob")
                nc.scalar.copy(ob[:, :], ps_o[:, :])
                nc.sync.dma_start(out_d[ss, :], ob[:, :])

            # position_bias output: strided DRAM->DRAM expansion (emitted
            # last so it never queues ahead of latency-critical transfers)
            for qi in range(NST):
                a = (S_ - 1) - 128 * qi
                (nc.sync if qi % 2 else nc.scalar).dma_start(
                    pb_d[qi * 128:(qi + 1) * 128, :], ctab[:, a:a + S_])

    nc.finalize()
    return nc


def _get_nc(S_=S):
    if ("nc", S_) not in _CACHE:
        _CACHE[("nc", S_)] = _build(S_)
    return _CACHE[("nc", S_)]


def _ensure_profile_hook():
    """Provide antenv.axon_hooks if the image lacks it, so trace=True can
    capture NTFF profiles (and a stray BASS_TRACE env cannot crash us)."""
    import types

    try:
        from antenv import axon_hooks  # noqa: F401
        return
    except ImportError:
        pass
    hook = None
    try:
        from trn_agent_boot.trn_boot import _ntff_profile_via_ctypes

        hook = _ntff_profile_via_ctypes("/opt/axon/libaxon_pjrt.so")
    except Exception:
        hook = None
    m = types.ModuleType("antenv.axon_hooks")
    m.get_axon_ntff_profile_hook = lambda: hook
    m.set_axon_ntff_profile_hook = lambda h: None
    sys.modules["antenv.axon_hooks"] = m


# ---------------------------------------------------------------- entry point
def kernel(hidden_states, mask, Wq, Wk, Wv, Wo, bias_emb):
    from concourse.bass_utils import run_bass_kernel_spmd

    hidden_states = np.asarray(hidden_states, dtype=np.float32)
    mask = np.asarray(mask, dtype=np.float32)
    Wq = np.ascontiguousarray(np.asarray(Wq, dtype=np.float16))
    Wk = np.ascontiguousarray(np.asarray(Wk, dtype=np.float16))
    Wv = np.ascontiguousarray(np.asarray(Wv, dtype=np.float16))
    Wo = np.ascontiguousarray(np.asarray(Wo, dtype=np.float16))
    bias_emb = np.asarray(bias_emb, dtype=np.float32)

    KC = S // 128
    gexp, ctab = _host_tables(bias_emb)
    nc = _get_nc()

    in_maps = []
    for c in range(B):
        in_maps.append({
            "hsT": np.ascontiguousarray(hidden_states[c].T.astype(np.float16)),
            "wq": Wq, "wk": Wk, "wv": Wv, "wo": Wo,
            "maskb": np.ascontiguousarray(
                mask[c, 0, 0, :].reshape(KC, 128).T + np.float32(SHIFT)),
            "gexp": gexp,
            "ctab": np.ascontiguousarray(ctab[c % H]),
        })

    trace = bool(int(os.environ.get("KERNEL_TRACE", "0")))
    _ensure_profile_hook()
    if trace:
        from concourse import bass_utils as _bu

        _bu.upload_artifacts = lambda tmpdir: tmpdir  # no artifact bucket here
    res = run_bass_kernel_spmd(nc, in_maps, core_ids=list(range(B)), trace=trace,
                               tmpdir=os.environ.get("KERNEL_TRACE_DIR"))
    global LAST_EXEC_NS, LAST_RESULT
    LAST_EXEC_NS = res.exec_time_ns
    LAST_RESULT = res

    out = np.stack([np.asarray(res.results[c]["out"]) for c in range(B)])
    pb = np.stack([np.asarray(res.results[c]["pb"]) for c in range(H)])[None]
    return out, pb


LAST_EXEC_NS = None
